# revision 10
# baseline (speedup 1.0000x reference)
"""ClosestPointLoss kernel for 8 trn2 NeuronCores — KD-pruned, scan-drained.

mean_i min_j ||outputs_i - targets_j||^2 over outputs [131072,3], targets [16384,3].

Host: KD-partition points into 1024 tiles ("slots") of 128; exact pruning
keeps ~190 of 16384 candidate targets per tile (upper bound from the 16
targets nearest each tile centroid; a target survives if its distance lower
bound to any 32-point sub-box beats that sub-box's bound). Verified exact
vs brute force.

Device: d^2(i,j) is a K=25 bf16 level-split matmul (rows: 3x |t|^2 levels,
18 cross-product rows, 3x |a|^2 levels, 1 offset row) — abs err ~5e-6.
Candidates are gathered into 128-col-padded slot blocks, packed into
2048-col PSUM groups (matmuls clipped at 512-col bank edges, alternating
two PE row bands). The whole group drains with ONE custom DVE op: an
inclusive prefix-min scan whose output AP is stride-0 within 128-element
pages, so each out column holds the scan value at that page end. A
per-slot additive offset (strictly increasing down the group, baked into
the offset matmul row) makes every later slot's values smaller than every
earlier slot's, so the scan value at a slot's last page IS that slot's
row-min; the host adds the offset back. This needs ~17 DVE ops per core
instead of one-per-slot (128+), sidestepping the ~350ns/op fixed cost.

Host epilogue: min over group-pieces per slot, sum, divide by N.
"""
import sys

sys.path.insert(0, "/opt/trn_rl_repo")

import numpy as np
from contextlib import ExitStack

N_CORES = 8
NPTS = 131072
NT = 16384
P_LEAF = 128          # points per slot (PE partition dim)
SUB = 16              # points per sub-box
NP_TILES = NPTS // P_LEAF   # 1024
NSLOT = NP_TILES // N_CORES # 128 slots per core
S_NEAR = 32           # targets per tile used for the UB bound
KROWS = 25            # matmul contraction rows (incl |a|^2 + offset rows)
GROUP = 2048          # cols per PSUM group (4 banks)
PAGE = 64             # scan output sampling page
CHUNK_GROUPS = 4      # R-streaming chunk size
PAD_VAL = np.float32(1e30)

PAIRS = [("hi", "hi"), ("hi", "lo"), ("lo", "hi"),
         ("hi", "l2"), ("l2", "hi"), ("lo", "lo")]

_compiled = {}


# ---------------------------------------------------------------- host math
def _kd_order(pts, leaf):
    out = []

    def rec(ids):
        if len(ids) <= leaf:
            out.append(ids)
            return
        p = pts[ids]
        ax = int(np.argmax(p.max(0) - p.min(0)))
        k = len(ids) // 2
        part = np.argpartition(p[:, ax], k)
        rec(ids[part[:k]])
        rec(ids[part[k:]])

    rec(np.arange(pts.shape[0]))
    return np.concatenate(out)


def _levels(x):
    import ml_dtypes
    bf = ml_dtypes.bfloat16
    hi = x.astype(bf).astype(np.float32)
    r = x - hi
    lo = r.astype(bf).astype(np.float32)
    l2 = (r - lo).astype(bf).astype(np.float32)
    return {"hi": hi, "lo": lo, "l2": l2}


def _candidates(outputs, targets):
    """KD order + exact per-tile candidate lists + per-tile max-dist bound D."""
    po = _kd_order(outputs, SUB)
    P = outputs[po].reshape(NP_TILES, P_LEAF, 3)
    Psub = outputs[po].reshape(NP_TILES, P_LEAF // SUB, SUB, 3)
    slo, shi = Psub.min(2), Psub.max(2)
    plo, phi = P.min(1), P.max(1)
    pc = 0.5 * (plo + phi)

    UBs = np.empty((NP_TILES, P_LEAF // SUB), np.float64)
    blk = 64
    for i0 in range(0, NP_TILES, blk):
        i1 = min(NP_TILES, i0 + blk)
        d_c = ((pc[i0:i1, None, :] - targets[None, :, :]) ** 2).sum(-1)
        S = np.argpartition(d_c, S_NEAR, axis=1)[:, :S_NEAR]
        ts = targets[S]                                   # [B,S,3]
        diff = Psub[i0:i1, :, :, None, :] - ts[:, None, None, :, :]
        dd = (diff ** 2).sum(-1)                          # [B,ns,SUB,S]
        UBs[i0:i1] = dd.min(3).max(2)

    cand, Dmax = [], np.empty(NP_TILES, np.float64)
    for i in range(NP_TILES):
        gap = np.maximum(0, np.maximum(targets[None, :, :] - shi[i][:, None, :],
                                       slo[i][:, None, :] - targets[None, :, :]))
        md2 = (gap ** 2).sum(-1)
        keep = (md2 <= UBs[i][:, None]).any(0)
        idx = np.nonzero(keep)[0]
        cand.append(idx)
        far = np.maximum(np.abs(targets[idx] - plo[i]),
                         np.abs(targets[idx] - phi[i]))
        Dmax[i] = (far ** 2).sum(-1).max()
    return po, cand, Dmax


def _schedule(cand):
    """Shared (core-independent) static schedule from the padded ladder."""
    cnt = np.array([len(c) for c in cand])
    cols = np.maximum(PAGE, -(-cnt // PAGE) * PAGE)      # 128-col padded
    order = np.argsort(-cols, kind="stable")             # ptile ids by work desc
    ladder = cols[order].reshape(NSLOT, N_CORES).max(1)  # [NSLOT] shared

    groups = []        # each: {'L', 'segs': [(psum_off, cols, r, slot_off, band)]}
    pieces = []        # (r, group_idx, end_pos, piece_cols) in stream order
    cur = {"L": 0, "segs": []}

    def close():
        nonlocal cur
        if cur["L"]:
            groups.append(cur)
            cur = {"L": 0, "segs": []}

    def cap():
        return 1024 if len(groups) < 4 else GROUP

    for r in range(NSLOT):
        rem = int(ladder[r])
        slot_off = 0
        while rem:
            if cur["L"] >= cap():
                close()
            take = min(rem, cap() - cur["L"])
            # emit segments clipped at 512-col bank edges
            p = cur["L"]
            left = take
            so = slot_off
            while left:
                seg = min(left, 512 - (p % 512))
                cur["segs"].append((p, seg, r, so))
                p += seg
                so += seg
                left -= seg
            pieces.append((r, len(groups), cur["L"] + take, take, slot_off))
            cur["L"] += take
            slot_off += take
            rem -= take
    close()

    pages0, np_ = [], 0
    for g in groups:
        pages0.append(np_)
        np_ += g["L"] // PAGE
    npages = np_

    # chunks of consecutive groups (first chunk = 1 group for a fast start)
    chunks = []
    bounds = [0, 2, 4]
    while bounds[-1] < len(groups):
        bounds.append(min(len(groups), bounds[-1] + CHUNK_GROUPS))
    bounds = sorted(set(min(b, len(groups)) for b in bounds))
    for c0, c1 in zip(bounds[:-1], bounds[1:]):
        gs = list(range(c0, c1))
        bcols = 0
        seg_rt = {}
        w_slots = set()
        for gi in gs:
            for (off, seg, r, so) in groups[gi]["segs"]:
                seg_rt[(gi, off)] = bcols
                bcols += seg
                w_slots.add(r)
        chunks.append({"groups": gs, "bcols": bcols, "rt": seg_rt,
                       "w_slots": (min(w_slots), max(w_slots))})
    CWB = sum(ch["bcols"] for ch in chunks)

    # per-slot sample list: (group, out_page_col)
    samples = {r: [] for r in range(NSLOT)}
    for k, (r, gi, end, pcols, soff) in enumerate(pieces):
        samples[r].append((gi, pages0[gi] + end // PAGE - 1, k))

    return dict(ladder=ladder, order=order, groups=groups, pieces=pieces,
                pages0=pages0, npages=npages, chunks=chunks, CWB=CWB,
                samples=samples)


def _build_operands(outputs, targets, po, cand, Dmax, sched):
    """Per-core W [50,NSLOT*128] / R [50,CWB] bf16 arrays + per-piece offsets."""
    import ml_dtypes
    bf = ml_dtypes.bfloat16

    U = (targets.astype(np.float64) ** 2).sum(1).astype(np.float32)
    Ulv = _levels(U)
    Tlv = _levels((-2.0 * targets.astype(np.float64)).astype(np.float32))
    Rfull = np.zeros((KROWS, NT), np.float32)
    Rfull[0], Rfull[1], Rfull[2] = Ulv["hi"], Ulv["lo"], Ulv["l2"]
    for p, (_, rl) in enumerate(PAIRS):
        Rfull[3 + 3 * p:6 + 3 * p] = Tlv[rl].T
    Rfull[21:24] = 1.0
    # row 24 (offset) set per-column during gather
    Rfull = Rfull.astype(bf).astype(np.float32)

    A = outputs[po].astype(np.float32)
    Alv = _levels(A)
    a2 = (outputs[po].astype(np.float64) ** 2).sum(1).astype(np.float32)
    a2lv = _levels(a2)
    Wfull = np.zeros((KROWS, NPTS), np.float32)
    Wfull[0:3] = 1.0
    for p, (wl, _) in enumerate(PAIRS):
        Wfull[3 + 3 * p:6 + 3 * p] = Alv[wl].T
    Wfull[21], Wfull[22], Wfull[23] = a2lv["hi"], a2lv["lo"], a2lv["l2"]
    Wfull[24] = 1.0
    Wfull = Wfull.astype(bf)

    order, ladder = sched["order"], sched["ladder"]
    groups, pieces, chunks = sched["groups"], sched["pieces"], sched["chunks"]

    W_dram = np.zeros((N_CORES, KROWS, NSLOT * P_LEAF), bf)
    R_dram = np.zeros((N_CORES, KROWS, sched["CWB"]), bf)
    offs = np.zeros((N_CORES, len(pieces)), np.float64)

    slot_ptile = np.empty((N_CORES, NSLOT), np.int64)
    for r in range(NSLOT):
        for c in range(N_CORES):
            pt = order[r * N_CORES + c]
            slot_ptile[c, r] = pt
            W_dram[c, :, r * P_LEAF:(r + 1) * P_LEAF] = \
                Wfull[:, pt * P_LEAF:(pt + 1) * P_LEAF]

    # per-core gathered candidate columns per slot (padded by replication)
    for c in range(N_CORES):
        slot_cols = {}
        for r in range(NSLOT):
            pt = slot_ptile[c, r]
            idx = cand[pt]
            n, padto = len(idx), int(ladder[r])
            idx = np.concatenate([idx, np.full(padto - n, idx[0])]) if n < padto else idx
            slot_cols[r] = Rfull[:, idx]          # [25, ladder[r]] f32

        # offsets per piece (reset each group, increasing within)
        piece_off = {}
        for gi in range(len(groups)):
            o = 0.0
            first = True
            for k, (r, g2, end, pcols, soff) in enumerate(pieces):
                if g2 != gi:
                    continue
                if not first:
                    o = o + np.ceil(Dmax[slot_ptile[c, r]]) + 1.0
                first = False
                piece_off[k] = o
                offs[c, k] = o
        assert max(piece_off.values()) <= 500, "offset overflow"

        # fill R: walk chunks/segments
        cw0 = 0
        for ch in chunks:
            for gi in ch["groups"]:
                for (off, seg, r, so) in groups[gi]["segs"]:
                    ok = [k for k, pc_ in enumerate(pieces)
                          if pc_[0] == r and pc_[1] == gi]
                    o = piece_off[ok[0]]
                    colblk = slot_cols[r][:, so:so + seg].copy()
                    colblk[24] = -o
                    rt = ch["rt"][(gi, off)]
                    R_dram[c, :, cw0 + rt:cw0 + rt + seg] = \
                        colblk.astype(R_dram.dtype)
            cw0 += ch["bcols"]
    return W_dram, R_dram, offs, slot_ptile


# ------------------------------------------------------------- device build
def _register_min_scan():
    from concourse import dve_ops
    from concourse.dve_ops import DveOp, OPS, _SUB_OPCODE_FOR_NAME, _CUSTOM_DVE_ROW_BASE
    from concourse.dve_spec import Spec, Src0, C0, Scan, minn, Zero

    if "MIN_SCAN_V1" in _SUB_OPCODE_FOR_NAME:
        return dve_ops.MIN_SCAN_V1

    MINOP = minn(Zero, Zero).op

    def _ref(in0, in1, c0, c1, c2):
        flat = in0.reshape(in0.shape[0], -1).astype(np.float32)
        sc = np.minimum.accumulate(flat, axis=-1)
        sc = np.minimum(sc, np.asarray(c0, np.float32).reshape(-1, 1))
        return sc.reshape(in0.shape)

    op = DveOp(
        "MIN_SCAN_V1",
        Spec(body=Scan(MINOP, Src0, init=C0), reference=_ref),
        subdim=False,
        uops_sha={},
    )
    from concourse.dve_ops import DveOpSpec, lower, has_src1

    for ver in ("v3", "v4"):
        spec = DveOpSpec(name=op.name, opcode=0, uops=lower(op.spec, ver=ver),
                         rd1_en=has_src1(op.spec))
        op.uops_sha[ver] = spec.sha(ver)
    OPS.append(op)
    _SUB_OPCODE_FOR_NAME[op.name] = _CUSTOM_DVE_ROW_BASE + len(OPS) - 1
    dve_ops.CUSTOM_DVE_SPECS[op.name] = op.spec
    dve_ops.MIN_SCAN_V1 = op
    return op


def _build(sched):
    import concourse.bacc as bacc
    import concourse.tile as tile
    from concourse import mybir

    MSC = _register_min_scan()
    f32 = mybir.dt.float32
    bf16 = mybir.dt.bfloat16

    groups, chunks = sched["groups"], sched["chunks"]
    npages, CWB = sched["npages"], sched["CWB"]

    nc = bacc.Bacc("TRN2", target_bir_lowering=False, debug=False)
    Wd = nc.dram_tensor("Wd", [KROWS, NSLOT * P_LEAF], bf16, kind="ExternalInput")
    Rd = nc.dram_tensor("Rd", [KROWS, CWB], bf16, kind="ExternalInput")
    out = nc.dram_tensor("out", [128, npages], f32, kind="ExternalOutput")

    with tile.TileContext(nc) as tc:
        with ExitStack() as ctx:
            singles = ctx.enter_context(tc.tile_pool(name="singles", bufs=1))
            Wsb = singles.tile([128, NSLOT * P_LEAF], bf16)
            out_sb = singles.tile([128, npages], f32)

            r_pool = ctx.enter_context(tc.tile_pool(name="rp", bufs=2))
            g_pool = ctx.enter_context(tc.tile_pool(name="gp", bufs=2, space="PSUM"))

            w_done = -1
            cw0 = 0
            for ch in chunks:
                w_lo, w_hi = ch["w_slots"]
                w_lo = max(w_lo, w_done + 1)
                if w_hi >= w_lo:
                    cs = slice(w_lo * P_LEAF, (w_hi + 1) * P_LEAF)
                    nc.sync.dma_start(out=Wsb[0:KROWS, cs], in_=Wd.ap()[:, cs])
                    w_done = w_hi
                bc = ch["bcols"]
                rt = r_pool.tile([128, bc], bf16, name="rt", tag="rt")
                nc.sync.dma_start(out=rt[0:KROWS, :],
                                  in_=Rd.ap()[:, cw0:cw0 + bc])

                for gi in ch["groups"]:
                    g = groups[gi]
                    L = g["L"]
                    gt = g_pool.tile([128, GROUP], f32, name="gt", tag="gt")
                    for (off, seg, r, so) in g["segs"]:
                        rto = ch["rt"][(gi, off)]
                        nc.tensor.matmul(
                            gt[:, off:off + seg],
                            Wsb[0:KROWS, r * P_LEAF:(r + 1) * P_LEAF],
                            rt[0:KROWS, rto:rto + seg],
                            start=True, stop=True, tile_position=(0, 0))
                    P = L // PAGE
                    p0 = sched["pages0"][gi]
                    in3 = gt[:, 0:L].rearrange("p (s o) -> p s o", o=PAGE)
                    out3 = out_sb[:, p0:p0 + P].rearrange(
                        "p (s o) -> p s o", o=1).broadcast_to((128, P, PAGE))
                    nc.vector._custom_dve(MSC, out=out3, in0=in3, s0=3.0e38)
                cw0 += ch["bcols"]

            nc.sync.dma_start(out=out.ap(), in_=out_sb[:, :])
    nc.compile()
    return nc


def _sched_key(sched):
    return (tuple(int(x) for x in sched["ladder"]), sched["CWB"], sched["npages"])


def _get_compiled(sched):
    key = _sched_key(sched)
    if key not in _compiled:
        _compiled[key] = _build(sched)
    return _compiled[key]


# ------------------------------------------------------------------- kernel
def kernel(outputs: np.ndarray, targets: np.ndarray) -> np.ndarray:
    from concourse.bass_utils import run_bass_kernel_spmd

    outputs = np.asarray(outputs, dtype=np.float32)
    targets = np.asarray(targets, dtype=np.float32)
    assert outputs.shape == (NPTS, 3) and targets.shape == (NT, 3)

    po, cand, Dmax = _candidates(outputs, targets)
    sched = _schedule(cand)
    W_dram, R_dram, offs, slot_ptile = _build_operands(
        outputs, targets, po, cand, Dmax, sched)

    nc = _get_compiled(sched)
    in_maps = [{"Wd": np.ascontiguousarray(W_dram[c]),
                "Rd": np.ascontiguousarray(R_dram[c])}
               for c in range(N_CORES)]
    res = run_bass_kernel_spmd(nc, in_maps, core_ids=list(range(N_CORES)))

    total = 0.0
    for c in range(N_CORES):
        o = res.results[c]["out"].astype(np.float64)
        for r in range(NSLOT):
            best = None
            for (gi, col, k) in sched["samples"][r]:
                v = o[:, col] + offs[c, k]
                best = v if best is None else np.minimum(best, v)
            total += best.sum()
    return np.float32(total / NPTS)


# revision 11
# speedup vs baseline: 1.0167x; 1.0167x over previous
"""ClosestPointLoss kernel for 8 trn2 NeuronCores — KD-pruned, scan-drained.

mean_i min_j ||outputs_i - targets_j||^2 over outputs [131072,3], targets [16384,3].

Host: KD-partition points into 1024 tiles ("slots") of 128; exact pruning
keeps ~190 of 16384 candidate targets per tile (upper bound from the 16
targets nearest each tile centroid; a target survives if its distance lower
bound to any 32-point sub-box beats that sub-box's bound). Verified exact
vs brute force.

Device: d^2(i,j) is a K=25 bf16 level-split matmul (rows: 3x |t|^2 levels,
18 cross-product rows, 3x |a|^2 levels, 1 offset row) — abs err ~5e-6.
Candidates are gathered into 128-col-padded slot blocks, packed into
2048-col PSUM groups (matmuls clipped at 512-col bank edges, alternating
two PE row bands). The whole group drains with ONE custom DVE op: an
inclusive prefix-min scan whose output AP is stride-0 within 128-element
pages, so each out column holds the scan value at that page end. A
per-slot additive offset (strictly increasing down the group, baked into
the offset matmul row) makes every later slot's values smaller than every
earlier slot's, so the scan value at a slot's last page IS that slot's
row-min; the host adds the offset back. This needs ~17 DVE ops per core
instead of one-per-slot (128+), sidestepping the ~350ns/op fixed cost.

Host epilogue: min over group-pieces per slot, sum, divide by N.
"""
import sys

sys.path.insert(0, "/opt/trn_rl_repo")

import numpy as np
from contextlib import ExitStack

N_CORES = 8
NPTS = 131072
NT = 16384
P_LEAF = 128          # points per slot (PE partition dim)
SUB = 16              # points per sub-box
NP_TILES = NPTS // P_LEAF   # 1024
NSLOT = NP_TILES // N_CORES # 128 slots per core
S_NEAR = 32           # targets per tile used for the UB bound
KROWS = 25            # matmul contraction rows (incl |a|^2 + offset rows)
GROUP = 2048          # cols per PSUM group (4 banks)
PAGE = 64             # scan output sampling page
CHUNK_GROUPS = 4      # R-streaming chunk size
PAD_VAL = np.float32(1e30)

PAIRS = [("hi", "hi"), ("hi", "lo"), ("lo", "hi"),
         ("hi", "l2"), ("l2", "hi"), ("lo", "lo")]

_compiled = {}


# ---------------------------------------------------------------- host math
def _kd_order(pts, leaf):
    out = []

    def rec(ids):
        if len(ids) <= leaf:
            out.append(ids)
            return
        p = pts[ids]
        ax = int(np.argmax(p.max(0) - p.min(0)))
        k = len(ids) // 2
        part = np.argpartition(p[:, ax], k)
        rec(ids[part[:k]])
        rec(ids[part[k:]])

    rec(np.arange(pts.shape[0]))
    return np.concatenate(out)


def _levels(x):
    import ml_dtypes
    bf = ml_dtypes.bfloat16
    hi = x.astype(bf).astype(np.float32)
    r = x - hi
    lo = r.astype(bf).astype(np.float32)
    l2 = (r - lo).astype(bf).astype(np.float32)
    return {"hi": hi, "lo": lo, "l2": l2}


def _candidates(outputs, targets):
    """KD order + exact per-tile candidate lists + per-tile max-dist bound D."""
    po = _kd_order(outputs, SUB)
    P = outputs[po].reshape(NP_TILES, P_LEAF, 3)
    Psub = outputs[po].reshape(NP_TILES, P_LEAF // SUB, SUB, 3)
    slo, shi = Psub.min(2), Psub.max(2)
    plo, phi = P.min(1), P.max(1)
    pc = 0.5 * (plo + phi)

    UBs = np.empty((NP_TILES, P_LEAF // SUB), np.float64)
    blk = 64
    for i0 in range(0, NP_TILES, blk):
        i1 = min(NP_TILES, i0 + blk)
        d_c = ((pc[i0:i1, None, :] - targets[None, :, :]) ** 2).sum(-1)
        S = np.argpartition(d_c, S_NEAR, axis=1)[:, :S_NEAR]
        ts = targets[S]                                   # [B,S,3]
        diff = Psub[i0:i1, :, :, None, :] - ts[:, None, None, :, :]
        dd = (diff ** 2).sum(-1)                          # [B,ns,SUB,S]
        UBs[i0:i1] = dd.min(3).max(2)

    cand, Dmax = [], np.empty(NP_TILES, np.float64)
    for i in range(NP_TILES):
        gap = np.maximum(0, np.maximum(targets[None, :, :] - shi[i][:, None, :],
                                       slo[i][:, None, :] - targets[None, :, :]))
        md2 = (gap ** 2).sum(-1)
        keep = (md2 <= UBs[i][:, None]).any(0)
        idx = np.nonzero(keep)[0]
        cand.append(idx)
        far = np.maximum(np.abs(targets[idx] - plo[i]),
                         np.abs(targets[idx] - phi[i]))
        Dmax[i] = (far ** 2).sum(-1).max()
    return po, cand, Dmax


def _schedule(cand):
    """Shared (core-independent) static schedule from the padded ladder."""
    cnt = np.array([len(c) for c in cand])
    cols = np.maximum(PAGE, -(-cnt // PAGE) * PAGE)      # 128-col padded
    order = np.argsort(-cols, kind="stable")             # ptile ids by work desc
    ladder = cols[order].reshape(NSLOT, N_CORES).max(1)  # [NSLOT] shared

    groups = []        # each: {'L', 'segs': [(psum_off, cols, r, slot_off, band)]}
    pieces = []        # (r, group_idx, end_pos, piece_cols) in stream order
    cur = {"L": 0, "segs": []}

    def close():
        nonlocal cur
        if cur["L"]:
            groups.append(cur)
            cur = {"L": 0, "segs": []}

    for r in range(NSLOT):
        rem = int(ladder[r])
        slot_off = 0
        while rem:
            if cur["L"] >= GROUP:
                close()
            take = min(rem, GROUP - cur["L"])
            # emit segments clipped at 512-col bank edges
            p = cur["L"]
            left = take
            so = slot_off
            while left:
                seg = min(left, 512 - (p % 512))
                cur["segs"].append((p, seg, r, so))
                p += seg
                so += seg
                left -= seg
            pieces.append((r, len(groups), cur["L"] + take, take, slot_off))
            cur["L"] += take
            slot_off += take
            rem -= take
    close()

    pages0, np_ = [], 0
    for g in groups:
        pages0.append(np_)
        np_ += g["L"] // PAGE
    npages = np_

    # chunks of consecutive groups (first chunk = 1 group for a fast start)
    chunks = []
    bounds = [0, 1, 3]
    while bounds[-1] < len(groups):
        bounds.append(min(len(groups), bounds[-1] + CHUNK_GROUPS))
    bounds = sorted(set(min(b, len(groups)) for b in bounds))
    for c0, c1 in zip(bounds[:-1], bounds[1:]):
        gs = list(range(c0, c1))
        bcols = 0
        seg_rt = {}
        w_slots = set()
        for gi in gs:
            for (off, seg, r, so) in groups[gi]["segs"]:
                seg_rt[(gi, off)] = bcols
                bcols += seg
                w_slots.add(r)
        chunks.append({"groups": gs, "bcols": bcols, "rt": seg_rt,
                       "w_slots": (min(w_slots), max(w_slots))})
    CWB = sum(ch["bcols"] for ch in chunks)

    # per-slot sample list: (group, out_page_col)
    samples = {r: [] for r in range(NSLOT)}
    for k, (r, gi, end, pcols, soff) in enumerate(pieces):
        samples[r].append((gi, pages0[gi] + end // PAGE - 1, k))

    return dict(ladder=ladder, order=order, groups=groups, pieces=pieces,
                pages0=pages0, npages=npages, chunks=chunks, CWB=CWB,
                samples=samples)


def _build_operands(outputs, targets, po, cand, Dmax, sched):
    """Per-core W [50,NSLOT*128] / R [50,CWB] bf16 arrays + per-piece offsets."""
    import ml_dtypes
    bf = ml_dtypes.bfloat16

    U = (targets.astype(np.float64) ** 2).sum(1).astype(np.float32)
    Ulv = _levels(U)
    Tlv = _levels((-2.0 * targets.astype(np.float64)).astype(np.float32))
    Rfull = np.zeros((KROWS, NT), np.float32)
    Rfull[0], Rfull[1], Rfull[2] = Ulv["hi"], Ulv["lo"], Ulv["l2"]
    for p, (_, rl) in enumerate(PAIRS):
        Rfull[3 + 3 * p:6 + 3 * p] = Tlv[rl].T
    Rfull[21:24] = 1.0
    # row 24 (offset) set per-column during gather
    Rfull = Rfull.astype(bf).astype(np.float32)

    A = outputs[po].astype(np.float32)
    Alv = _levels(A)
    a2 = (outputs[po].astype(np.float64) ** 2).sum(1).astype(np.float32)
    a2lv = _levels(a2)
    Wfull = np.zeros((KROWS, NPTS), np.float32)
    Wfull[0:3] = 1.0
    for p, (wl, _) in enumerate(PAIRS):
        Wfull[3 + 3 * p:6 + 3 * p] = Alv[wl].T
    Wfull[21], Wfull[22], Wfull[23] = a2lv["hi"], a2lv["lo"], a2lv["l2"]
    Wfull[24] = 1.0
    Wfull = Wfull.astype(bf)

    order, ladder = sched["order"], sched["ladder"]
    groups, pieces, chunks = sched["groups"], sched["pieces"], sched["chunks"]

    W_dram = np.zeros((N_CORES, KROWS, NSLOT * P_LEAF), bf)
    R_dram = np.zeros((N_CORES, KROWS, sched["CWB"]), bf)
    offs = np.zeros((N_CORES, len(pieces)), np.float64)

    slot_ptile = np.empty((N_CORES, NSLOT), np.int64)
    for r in range(NSLOT):
        for c in range(N_CORES):
            pt = order[r * N_CORES + c]
            slot_ptile[c, r] = pt
            W_dram[c, :, r * P_LEAF:(r + 1) * P_LEAF] = \
                Wfull[:, pt * P_LEAF:(pt + 1) * P_LEAF]

    # per-core gathered candidate columns per slot (padded by replication)
    for c in range(N_CORES):
        slot_cols = {}
        for r in range(NSLOT):
            pt = slot_ptile[c, r]
            idx = cand[pt]
            n, padto = len(idx), int(ladder[r])
            idx = np.concatenate([idx, np.full(padto - n, idx[0])]) if n < padto else idx
            slot_cols[r] = Rfull[:, idx]          # [25, ladder[r]] f32

        # offsets per piece (reset each group, increasing within)
        piece_off = {}
        for gi in range(len(groups)):
            o = 0.0
            first = True
            for k, (r, g2, end, pcols, soff) in enumerate(pieces):
                if g2 != gi:
                    continue
                if not first:
                    o = o + np.ceil(Dmax[slot_ptile[c, r]]) + 1.0
                first = False
                piece_off[k] = o
                offs[c, k] = o
        assert max(piece_off.values()) <= 500, "offset overflow"

        # fill R: walk chunks/segments
        cw0 = 0
        for ch in chunks:
            for gi in ch["groups"]:
                for (off, seg, r, so) in groups[gi]["segs"]:
                    ok = [k for k, pc_ in enumerate(pieces)
                          if pc_[0] == r and pc_[1] == gi]
                    o = piece_off[ok[0]]
                    colblk = slot_cols[r][:, so:so + seg].copy()
                    colblk[24] = -o
                    rt = ch["rt"][(gi, off)]
                    R_dram[c, :, cw0 + rt:cw0 + rt + seg] = \
                        colblk.astype(R_dram.dtype)
            cw0 += ch["bcols"]
    return W_dram, R_dram, offs, slot_ptile


# ------------------------------------------------------------- device build
def _register_min_scan():
    from concourse import dve_ops
    from concourse.dve_ops import DveOp, OPS, _SUB_OPCODE_FOR_NAME, _CUSTOM_DVE_ROW_BASE
    from concourse.dve_spec import Spec, Src0, C0, Scan, minn, Zero

    if "MIN_SCAN_V1" in _SUB_OPCODE_FOR_NAME:
        return dve_ops.MIN_SCAN_V1

    MINOP = minn(Zero, Zero).op

    def _ref(in0, in1, c0, c1, c2):
        flat = in0.reshape(in0.shape[0], -1).astype(np.float32)
        sc = np.minimum.accumulate(flat, axis=-1)
        sc = np.minimum(sc, np.asarray(c0, np.float32).reshape(-1, 1))
        return sc.reshape(in0.shape)

    op = DveOp(
        "MIN_SCAN_V1",
        Spec(body=Scan(MINOP, Src0, init=C0), reference=_ref),
        subdim=False,
        uops_sha={},
    )
    from concourse.dve_ops import DveOpSpec, lower, has_src1

    for ver in ("v3", "v4"):
        spec = DveOpSpec(name=op.name, opcode=0, uops=lower(op.spec, ver=ver),
                         rd1_en=has_src1(op.spec))
        op.uops_sha[ver] = spec.sha(ver)
    OPS.append(op)
    _SUB_OPCODE_FOR_NAME[op.name] = _CUSTOM_DVE_ROW_BASE + len(OPS) - 1
    dve_ops.CUSTOM_DVE_SPECS[op.name] = op.spec
    dve_ops.MIN_SCAN_V1 = op
    return op


def _build(sched):
    import concourse.bacc as bacc
    import concourse.tile as tile
    from concourse import mybir

    MSC = _register_min_scan()
    f32 = mybir.dt.float32
    bf16 = mybir.dt.bfloat16

    groups, chunks = sched["groups"], sched["chunks"]
    npages, CWB = sched["npages"], sched["CWB"]

    nc = bacc.Bacc("TRN2", target_bir_lowering=False, debug=False)
    Wd = nc.dram_tensor("Wd", [KROWS, NSLOT * P_LEAF], bf16, kind="ExternalInput")
    Rd = nc.dram_tensor("Rd", [KROWS, CWB], bf16, kind="ExternalInput")
    out = nc.dram_tensor("out", [128, npages], f32, kind="ExternalOutput")

    with tile.TileContext(nc) as tc:
        with ExitStack() as ctx:
            singles = ctx.enter_context(tc.tile_pool(name="singles", bufs=1))
            Wsb = singles.tile([128, NSLOT * P_LEAF], bf16)
            out_sb = singles.tile([128, npages], f32)

            r_pool = ctx.enter_context(tc.tile_pool(name="rp", bufs=2))
            g_pool = ctx.enter_context(tc.tile_pool(name="gp", bufs=2, space="PSUM"))

            w_done = -1
            cw0 = 0
            for ch in chunks:
                w_lo, w_hi = ch["w_slots"]
                w_lo = max(w_lo, w_done + 1)
                if w_hi >= w_lo:
                    cs = slice(w_lo * P_LEAF, (w_hi + 1) * P_LEAF)
                    nc.sync.dma_start(out=Wsb[0:KROWS, cs], in_=Wd.ap()[:, cs])
                    w_done = w_hi
                bc = ch["bcols"]
                rt = r_pool.tile([128, bc], bf16, name="rt", tag="rt")
                nc.sync.dma_start(out=rt[0:KROWS, :],
                                  in_=Rd.ap()[:, cw0:cw0 + bc])

                for gi in ch["groups"]:
                    g = groups[gi]
                    L = g["L"]
                    gt = g_pool.tile([128, GROUP], f32, name="gt", tag="gt")
                    for (off, seg, r, so) in g["segs"]:
                        rto = ch["rt"][(gi, off)]
                        nc.tensor.matmul(
                            gt[:, off:off + seg],
                            Wsb[0:KROWS, r * P_LEAF:(r + 1) * P_LEAF],
                            rt[0:KROWS, rto:rto + seg],
                            start=True, stop=True, tile_position=(0, 0))
                    P = L // PAGE
                    p0 = sched["pages0"][gi]
                    in3 = gt[:, 0:L].rearrange("p (s o) -> p s o", o=PAGE)
                    out3 = out_sb[:, p0:p0 + P].rearrange(
                        "p (s o) -> p s o", o=1).broadcast_to((128, P, PAGE))
                    nc.vector._custom_dve(MSC, out=out3, in0=in3, s0=3.0e38)
                cw0 += ch["bcols"]

            nc.sync.dma_start(out=out.ap(), in_=out_sb[:, :])
    nc.compile()
    return nc


def _sched_key(sched):
    return (tuple(int(x) for x in sched["ladder"]), sched["CWB"], sched["npages"])


def _get_compiled(sched):
    key = _sched_key(sched)
    if key not in _compiled:
        _compiled[key] = _build(sched)
    return _compiled[key]


# ------------------------------------------------------------------- kernel
def kernel(outputs: np.ndarray, targets: np.ndarray) -> np.ndarray:
    from concourse.bass_utils import run_bass_kernel_spmd

    outputs = np.asarray(outputs, dtype=np.float32)
    targets = np.asarray(targets, dtype=np.float32)
    assert outputs.shape == (NPTS, 3) and targets.shape == (NT, 3)

    po, cand, Dmax = _candidates(outputs, targets)
    sched = _schedule(cand)
    W_dram, R_dram, offs, slot_ptile = _build_operands(
        outputs, targets, po, cand, Dmax, sched)

    nc = _get_compiled(sched)
    in_maps = [{"Wd": np.ascontiguousarray(W_dram[c]),
                "Rd": np.ascontiguousarray(R_dram[c])}
               for c in range(N_CORES)]
    res = run_bass_kernel_spmd(nc, in_maps, core_ids=list(range(N_CORES)))

    total = 0.0
    for c in range(N_CORES):
        o = res.results[c]["out"].astype(np.float64)
        for r in range(NSLOT):
            best = None
            for (gi, col, k) in sched["samples"][r]:
                v = o[:, col] + offs[c, k]
                best = v if best is None else np.minimum(best, v)
            total += best.sum()
    return np.float32(total / NPTS)


# revision 12
# speedup vs baseline: 1.0592x; 1.0418x over previous
"""ClosestPointLoss kernel for 8 trn2 NeuronCores — KD-pruned, scan-drained.

mean_i min_j ||outputs_i - targets_j||^2 over outputs [131072,3], targets [16384,3].

Host: KD-partition points into 1024 tiles ("slots") of 128; exact pruning
keeps ~190 of 16384 candidate targets per tile (upper bound from the 16
targets nearest each tile centroid; a target survives if its distance lower
bound to any 32-point sub-box beats that sub-box's bound). Verified exact
vs brute force.

Device: d^2(i,j) is a K=25 bf16 level-split matmul (rows: 3x |t|^2 levels,
18 cross-product rows, 3x |a|^2 levels, 1 offset row) — abs err ~5e-6.
Candidates are gathered into 128-col-padded slot blocks, packed into
2048-col PSUM groups (matmuls clipped at 512-col bank edges, alternating
two PE row bands). The whole group drains with ONE custom DVE op: an
inclusive prefix-min scan whose output AP is stride-0 within 128-element
pages, so each out column holds the scan value at that page end. A
per-slot additive offset (strictly increasing down the group, baked into
the offset matmul row) makes every later slot's values smaller than every
earlier slot's, so the scan value at a slot's last page IS that slot's
row-min; the host adds the offset back. This needs ~17 DVE ops per core
instead of one-per-slot (128+), sidestepping the ~350ns/op fixed cost.

Host epilogue: min over group-pieces per slot, sum, divide by N.
"""
import sys

sys.path.insert(0, "/opt/trn_rl_repo")

import numpy as np
from contextlib import ExitStack

N_CORES = 8
NPTS = 131072
NT = 16384
P_LEAF = 128          # points per slot (PE partition dim)
SUB = 16              # points per sub-box
NP_TILES = NPTS // P_LEAF   # 1024
NSLOT = NP_TILES // N_CORES # 128 slots per core
S_NEAR = 64           # targets per tile used for the UB bound
KROWS = 25            # matmul contraction rows (incl |a|^2 + offset rows)
GROUP = 2048          # cols per PSUM group (4 banks)
PAGE = 64             # scan output sampling page
CHUNK_GROUPS = 8      # R-streaming chunk size
PAD_VAL = np.float32(1e30)

PAIRS = [("hi", "hi"), ("hi", "lo"), ("lo", "hi"),
         ("hi", "l2"), ("l2", "hi"), ("lo", "lo")]

_compiled = {}


# ---------------------------------------------------------------- host math
def _kd_order(pts, leaf):
    out = []

    def rec(ids):
        if len(ids) <= leaf:
            out.append(ids)
            return
        p = pts[ids]
        ax = int(np.argmax(p.max(0) - p.min(0)))
        k = len(ids) // 2
        part = np.argpartition(p[:, ax], k)
        rec(ids[part[:k]])
        rec(ids[part[k:]])

    rec(np.arange(pts.shape[0]))
    return np.concatenate(out)


def _levels(x):
    import ml_dtypes
    bf = ml_dtypes.bfloat16
    hi = x.astype(bf).astype(np.float32)
    r = x - hi
    lo = r.astype(bf).astype(np.float32)
    l2 = (r - lo).astype(bf).astype(np.float32)
    return {"hi": hi, "lo": lo, "l2": l2}


def _candidates(outputs, targets):
    """KD order + exact per-tile candidate lists + per-tile max-dist bound D."""
    po = _kd_order(outputs, SUB)
    P = outputs[po].reshape(NP_TILES, P_LEAF, 3)
    Psub = outputs[po].reshape(NP_TILES, P_LEAF // SUB, SUB, 3)
    slo, shi = Psub.min(2), Psub.max(2)
    plo, phi = P.min(1), P.max(1)
    pc = 0.5 * (plo + phi)

    UBs = np.empty((NP_TILES, P_LEAF // SUB), np.float64)
    blk = 64
    for i0 in range(0, NP_TILES, blk):
        i1 = min(NP_TILES, i0 + blk)
        d_c = ((pc[i0:i1, None, :] - targets[None, :, :]) ** 2).sum(-1)
        S = np.argpartition(d_c, S_NEAR, axis=1)[:, :S_NEAR]
        ts = targets[S]                                   # [B,S,3]
        diff = Psub[i0:i1, :, :, None, :] - ts[:, None, None, :, :]
        dd = (diff ** 2).sum(-1)                          # [B,ns,SUB,S]
        UBs[i0:i1] = dd.min(3).max(2)

    cand, Dmax = [], np.empty(NP_TILES, np.float64)
    for i in range(NP_TILES):
        gap = np.maximum(0, np.maximum(targets[None, :, :] - shi[i][:, None, :],
                                       slo[i][:, None, :] - targets[None, :, :]))
        md2 = (gap ** 2).sum(-1)
        keep = (md2 <= UBs[i][:, None]).any(0)
        idx = np.nonzero(keep)[0]
        cand.append(idx)
        far = np.maximum(np.abs(targets[idx] - plo[i]),
                         np.abs(targets[idx] - phi[i]))
        Dmax[i] = (far ** 2).sum(-1).max()
    return po, cand, Dmax


def _schedule(cand):
    """Shared (core-independent) static schedule from the padded ladder."""
    cnt = np.array([len(c) for c in cand])
    cols = np.maximum(PAGE, -(-cnt // PAGE) * PAGE)      # 128-col padded
    order = np.argsort(-cols, kind="stable")             # ptile ids by work desc
    ladder = cols[order].reshape(NSLOT, N_CORES).max(1)  # [NSLOT] shared

    groups = []        # each: {'L', 'segs': [(psum_off, cols, r, slot_off, band)]}
    pieces = []        # (r, group_idx, end_pos, piece_cols) in stream order
    cur = {"L": 0, "segs": []}

    def close():
        nonlocal cur
        if cur["L"]:
            groups.append(cur)
            cur = {"L": 0, "segs": []}

    for r in range(NSLOT):
        rem = int(ladder[r])
        slot_off = 0
        while rem:
            if cur["L"] >= GROUP:
                close()
            take = min(rem, GROUP - cur["L"])
            # emit segments clipped at 512-col bank edges
            p = cur["L"]
            left = take
            so = slot_off
            while left:
                seg = min(left, 512 - (p % 512))
                cur["segs"].append((p, seg, r, so))
                p += seg
                so += seg
                left -= seg
            pieces.append((r, len(groups), cur["L"] + take, take, slot_off))
            cur["L"] += take
            slot_off += take
            rem -= take
    close()

    pages0, np_ = [], 0
    for g in groups:
        pages0.append(np_)
        np_ += g["L"] // PAGE
    npages = np_

    # chunks of consecutive groups (first chunk = 1 group for a fast start)
    chunks = []
    bounds = [0, 1, 3]
    while bounds[-1] < len(groups):
        bounds.append(min(len(groups), bounds[-1] + CHUNK_GROUPS))
    bounds = sorted(set(min(b, len(groups)) for b in bounds))
    for c0, c1 in zip(bounds[:-1], bounds[1:]):
        gs = list(range(c0, c1))
        bcols = 0
        seg_rt = {}
        w_slots = set()
        for gi in gs:
            for (off, seg, r, so) in groups[gi]["segs"]:
                seg_rt[(gi, off)] = bcols
                bcols += seg
                w_slots.add(r)
        chunks.append({"groups": gs, "bcols": bcols, "rt": seg_rt,
                       "w_slots": (min(w_slots), max(w_slots))})
    CWB = sum(ch["bcols"] for ch in chunks)

    # per-slot sample list: (group, out_page_col)
    samples = {r: [] for r in range(NSLOT)}
    for k, (r, gi, end, pcols, soff) in enumerate(pieces):
        samples[r].append((gi, pages0[gi] + end // PAGE - 1, k))

    return dict(ladder=ladder, order=order, groups=groups, pieces=pieces,
                pages0=pages0, npages=npages, chunks=chunks, CWB=CWB,
                samples=samples)


def _build_operands(outputs, targets, po, cand, Dmax, sched):
    """Per-core W [50,NSLOT*128] / R [50,CWB] bf16 arrays + per-piece offsets."""
    import ml_dtypes
    bf = ml_dtypes.bfloat16

    U = (targets.astype(np.float64) ** 2).sum(1).astype(np.float32)
    Ulv = _levels(U)
    Tlv = _levels((-2.0 * targets.astype(np.float64)).astype(np.float32))
    Rfull = np.zeros((KROWS, NT), np.float32)
    Rfull[0], Rfull[1], Rfull[2] = Ulv["hi"], Ulv["lo"], Ulv["l2"]
    for p, (_, rl) in enumerate(PAIRS):
        Rfull[3 + 3 * p:6 + 3 * p] = Tlv[rl].T
    Rfull[21:24] = 1.0
    # row 24 (offset) set per-column during gather
    Rfull = Rfull.astype(bf).astype(np.float32)

    A = outputs[po].astype(np.float32)
    Alv = _levels(A)
    a2 = (outputs[po].astype(np.float64) ** 2).sum(1).astype(np.float32)
    a2lv = _levels(a2)
    Wfull = np.zeros((KROWS, NPTS), np.float32)
    Wfull[0:3] = 1.0
    for p, (wl, _) in enumerate(PAIRS):
        Wfull[3 + 3 * p:6 + 3 * p] = Alv[wl].T
    Wfull[21], Wfull[22], Wfull[23] = a2lv["hi"], a2lv["lo"], a2lv["l2"]
    Wfull[24] = 1.0
    Wfull = Wfull.astype(bf)

    order, ladder = sched["order"], sched["ladder"]
    groups, pieces, chunks = sched["groups"], sched["pieces"], sched["chunks"]

    W_dram = np.zeros((N_CORES, KROWS, NSLOT * P_LEAF), bf)
    R_dram = np.zeros((N_CORES, KROWS, sched["CWB"]), bf)
    offs = np.zeros((N_CORES, len(pieces)), np.float64)

    slot_ptile = np.empty((N_CORES, NSLOT), np.int64)
    for r in range(NSLOT):
        for c in range(N_CORES):
            pt = order[r * N_CORES + c]
            slot_ptile[c, r] = pt
            W_dram[c, :, r * P_LEAF:(r + 1) * P_LEAF] = \
                Wfull[:, pt * P_LEAF:(pt + 1) * P_LEAF]

    # per-core gathered candidate columns per slot (padded by replication)
    for c in range(N_CORES):
        slot_cols = {}
        for r in range(NSLOT):
            pt = slot_ptile[c, r]
            idx = cand[pt]
            n, padto = len(idx), int(ladder[r])
            idx = np.concatenate([idx, np.full(padto - n, idx[0])]) if n < padto else idx
            slot_cols[r] = Rfull[:, idx]          # [25, ladder[r]] f32

        # offsets per piece (reset each group, increasing within)
        piece_off = {}
        for gi in range(len(groups)):
            o = 0.0
            first = True
            for k, (r, g2, end, pcols, soff) in enumerate(pieces):
                if g2 != gi:
                    continue
                if not first:
                    o = o + np.ceil(Dmax[slot_ptile[c, r]]) + 1.0
                first = False
                piece_off[k] = o
                offs[c, k] = o
        assert max(piece_off.values()) <= 500, "offset overflow"

        # fill R: walk chunks/segments
        cw0 = 0
        for ch in chunks:
            for gi in ch["groups"]:
                for (off, seg, r, so) in groups[gi]["segs"]:
                    ok = [k for k, pc_ in enumerate(pieces)
                          if pc_[0] == r and pc_[1] == gi]
                    o = piece_off[ok[0]]
                    colblk = slot_cols[r][:, so:so + seg].copy()
                    colblk[24] = -o
                    rt = ch["rt"][(gi, off)]
                    R_dram[c, :, cw0 + rt:cw0 + rt + seg] = \
                        colblk.astype(R_dram.dtype)
            cw0 += ch["bcols"]
    return W_dram, R_dram, offs, slot_ptile


# ------------------------------------------------------------- device build
def _register_min_scan():
    from concourse import dve_ops
    from concourse.dve_ops import DveOp, OPS, _SUB_OPCODE_FOR_NAME, _CUSTOM_DVE_ROW_BASE
    from concourse.dve_spec import Spec, Src0, C0, Scan, minn, Zero

    if "MIN_SCAN_V1" in _SUB_OPCODE_FOR_NAME:
        return dve_ops.MIN_SCAN_V1

    MINOP = minn(Zero, Zero).op

    def _ref(in0, in1, c0, c1, c2):
        flat = in0.reshape(in0.shape[0], -1).astype(np.float32)
        sc = np.minimum.accumulate(flat, axis=-1)
        sc = np.minimum(sc, np.asarray(c0, np.float32).reshape(-1, 1))
        return sc.reshape(in0.shape)

    op = DveOp(
        "MIN_SCAN_V1",
        Spec(body=Scan(MINOP, Src0, init=C0), reference=_ref),
        subdim=False,
        uops_sha={},
    )
    from concourse.dve_ops import DveOpSpec, lower, has_src1

    for ver in ("v3", "v4"):
        spec = DveOpSpec(name=op.name, opcode=0, uops=lower(op.spec, ver=ver),
                         rd1_en=has_src1(op.spec))
        op.uops_sha[ver] = spec.sha(ver)
    OPS.append(op)
    _SUB_OPCODE_FOR_NAME[op.name] = _CUSTOM_DVE_ROW_BASE + len(OPS) - 1
    dve_ops.CUSTOM_DVE_SPECS[op.name] = op.spec
    dve_ops.MIN_SCAN_V1 = op
    return op


def _build(sched):
    import concourse.bacc as bacc
    import concourse.tile as tile
    from concourse import mybir

    MSC = _register_min_scan()
    f32 = mybir.dt.float32
    bf16 = mybir.dt.bfloat16

    groups, chunks = sched["groups"], sched["chunks"]
    npages, CWB = sched["npages"], sched["CWB"]

    nc = bacc.Bacc("TRN2", target_bir_lowering=False, debug=False)
    Wd = nc.dram_tensor("Wd", [KROWS, NSLOT * P_LEAF], bf16, kind="ExternalInput")
    Rd = nc.dram_tensor("Rd", [KROWS, CWB], bf16, kind="ExternalInput")
    out = nc.dram_tensor("out", [128, npages], f32, kind="ExternalOutput")

    with tile.TileContext(nc) as tc:
        with ExitStack() as ctx:
            singles = ctx.enter_context(tc.tile_pool(name="singles", bufs=1))
            Wsb = singles.tile([128, NSLOT * P_LEAF], bf16)
            out_sb = singles.tile([128, npages], f32)

            r_pool = ctx.enter_context(tc.tile_pool(name="rp", bufs=2))
            g_pool = ctx.enter_context(tc.tile_pool(name="gp", bufs=2, space="PSUM"))

            w_done = -1
            cw0 = 0
            for ch in chunks:
                w_lo, w_hi = ch["w_slots"]
                w_lo = max(w_lo, w_done + 1)
                if w_hi >= w_lo:
                    cs = slice(w_lo * P_LEAF, (w_hi + 1) * P_LEAF)
                    nc.sync.dma_start(out=Wsb[0:KROWS, cs], in_=Wd.ap()[:, cs])
                    w_done = w_hi
                bc = ch["bcols"]
                rt = r_pool.tile([128, bc], bf16, name="rt", tag="rt")
                nc.sync.dma_start(out=rt[0:KROWS, :],
                                  in_=Rd.ap()[:, cw0:cw0 + bc])

                for gi in ch["groups"]:
                    g = groups[gi]
                    L = g["L"]
                    gt = g_pool.tile([128, GROUP], f32, name="gt", tag="gt")
                    for (off, seg, r, so) in g["segs"]:
                        rto = ch["rt"][(gi, off)]
                        nc.tensor.matmul(
                            gt[:, off:off + seg],
                            Wsb[0:KROWS, r * P_LEAF:(r + 1) * P_LEAF],
                            rt[0:KROWS, rto:rto + seg],
                            start=True, stop=True, tile_position=(0, 0))
                    P = L // PAGE
                    p0 = sched["pages0"][gi]
                    in3 = gt[:, 0:L].rearrange("p (s o) -> p s o", o=PAGE)
                    out3 = out_sb[:, p0:p0 + P].rearrange(
                        "p (s o) -> p s o", o=1).broadcast_to((128, P, PAGE))
                    nc.vector._custom_dve(MSC, out=out3, in0=in3, s0=3.0e38)
                cw0 += ch["bcols"]

            nc.sync.dma_start(out=out.ap(), in_=out_sb[:, :])
    nc.compile()
    return nc


def _sched_key(sched):
    return (tuple(int(x) for x in sched["ladder"]), sched["CWB"], sched["npages"])


def _get_compiled(sched):
    key = _sched_key(sched)
    if key not in _compiled:
        _compiled[key] = _build(sched)
    return _compiled[key]


# ------------------------------------------------------------------- kernel
def kernel(outputs: np.ndarray, targets: np.ndarray) -> np.ndarray:
    from concourse.bass_utils import run_bass_kernel_spmd

    outputs = np.asarray(outputs, dtype=np.float32)
    targets = np.asarray(targets, dtype=np.float32)
    assert outputs.shape == (NPTS, 3) and targets.shape == (NT, 3)

    po, cand, Dmax = _candidates(outputs, targets)
    sched = _schedule(cand)
    W_dram, R_dram, offs, slot_ptile = _build_operands(
        outputs, targets, po, cand, Dmax, sched)

    nc = _get_compiled(sched)
    in_maps = [{"Wd": np.ascontiguousarray(W_dram[c]),
                "Rd": np.ascontiguousarray(R_dram[c])}
               for c in range(N_CORES)]
    res = run_bass_kernel_spmd(nc, in_maps, core_ids=list(range(N_CORES)))

    total = 0.0
    for c in range(N_CORES):
        o = res.results[c]["out"].astype(np.float64)
        for r in range(NSLOT):
            best = None
            for (gi, col, k) in sched["samples"][r]:
                v = o[:, col] + offs[c, k]
                best = v if best is None else np.minimum(best, v)
            total += best.sum()
    return np.float32(total / NPTS)
